# revision 4
# baseline (speedup 1.0000x reference)
"""Trainium2 Bass kernel for per-edge-type linear routing (MoE-style).

Computes out[i] = W[type_i] @ x[i] + b[type_i] for N=131072 edges,
C=D=256, T=8 types, on 8 NeuronCores.

Strategy: expert-grouped data parallelism in bf16. On the host we
stable-sort the edges by type and deal them round-robin to the 8 cores,
so every core gets ~N/8 edges grouped into 8 contiguous per-type
segments (segment sizes shared across cores -> identical SPMD program).
Inputs, weights and outputs move as bf16 (the PSUM accumulation stays
fp32), which halves both the HBM traffic and the PE time versus fp32:
bf16 matmuls stream 1 column/cycle vs fp32's 4.

Per core the weights are STATIONARY: for each type segment the four
128x128 blocks of W[t]^T sit in the PE array while the edge features
stream through as the moving operand, so LDWEIGHTS is amortized over a
whole segment and segments need no 128-column padding (the moving free
dim is arbitrary). Layout per core:

  xt   [2, 128, NP] bf16  edge features^T, type-grouped (ch-half, c, col)
  w    [128, 4096]  bf16  w[c, ((t*2+ch)*2+dh)*128 + dc] = W[t, dh*128+dc, ch*128+c]
  bias [128, 16]    f32   bias[p, t*2+dh] = b[t, dh*128+p]
  y    [2, 128, NP]  bf16  outputs^T in the same grouped order

PSUM->SBUF evict adds the per-partition bias and downcasts to bf16;
evictions alternate between the DVE (dh=0) and ACT (dh=1) engines so
they stay off the critical path. The host scatters valid columns of y
back to the original edge order in fp32.
"""

import numpy as np

N_CORES = 8
T = 8
C = 256
D = 256
P = 128
ALIGN = 16   # segment length quantum (32B DMA alignment in bf16)
GRP = 1024   # edge columns per PSUM group (2 banks)
SUB = 512    # moving columns per matmul (1 PSUM bank of fp32)

_cache = {}


def _build_program(G, R=1):
    """Build + compile the SPMD Bass program for one core.

    G: tuple of per-type segment lengths (each a multiple of ALIGN).
    R: number of times the whole kernel body is unrolled (R>1 is used
       by the timing harness to measure steady-state per-iteration time).
    """
    import concourse.tile as tile
    from concourse import bacc, mybir

    f32 = mybir.dt.float32
    bf16 = mybir.dt.bfloat16
    add = mybir.AluOpType.add
    copy_fn = mybir.ActivationFunctionType.Identity

    NP = int(sum(G))
    Gmax = int(max(G))
    offs = np.concatenate([[0], np.cumsum(G)]).astype(np.int64)

    nc = bacc.Bacc("TRN2", target_bir_lowering=False, debug=False)

    xt = nc.dram_tensor("xt", [2, P, NP], bf16, kind="ExternalInput")
    w = nc.dram_tensor("w", [P, T * 4 * P], bf16, kind="ExternalInput")
    bias = nc.dram_tensor("bias", [P, 2 * T], f32, kind="ExternalInput")
    chain = nc.dram_tensor("chain", [1, 4], f32, kind="ExternalInput")
    y = nc.dram_tensor("y", [2, P, NP], bf16, kind="ExternalOutput")
    chain_out = nc.dram_tensor("chain_out", [1, 4], f32, kind="ExternalOutput")

    with tile.TileContext(nc) as tc:
        with (
            tc.tile_pool(name="wpool", bufs=2) as wpool,
            tc.tile_pool(name="xpool", bufs=4) as xpool,
            tc.tile_pool(name="ypool", bufs=4) as ypool,
            tc.tile_pool(name="pspool", bufs=2, space="PSUM") as pspool,
        ):
            for rep in range(R):
                # Tiny passthrough so a timing harness can chain executions.
                cht = wpool.tile([1, 4], f32, name="cht", tag="cht")
                nc.sync.dma_start(out=cht[:], in_=chain[:])
                nc.sync.dma_start(out=chain_out[:], in_=cht[:])

                wsb = wpool.tile([P, T * 4 * P], bf16, name="wsb", tag="wsb")
                nc.sync.dma_start(out=wsb[:], in_=w[:])
                bsb = wpool.tile([P, 2 * T], f32, name="bsb", tag="bsb")
                nc.sync.dma_start(out=bsb[:], in_=bias[:])

                for t in range(T):
                    Gt = int(G[t])
                    if Gt == 0:
                        continue
                    off = int(offs[t])
                    xh = []
                    for ch in range(2):
                        xs = xpool.tile([P, Gmax], bf16, name=f"x{ch}",
                                        tag=f"x{ch}")
                        nc.sync.dma_start(
                            out=xs[:, :Gt], in_=xt[ch, :, off:off + Gt]
                        )
                        xh.append(xs)
                    yh = [
                        ypool.tile([P, Gmax], bf16, name=f"y{dh}", tag=f"y{dh}")
                        for dh in range(2)
                    ]
                    for g0 in range(0, Gt, GRP):
                        Eg = min(GRP, Gt - g0)
                        for dh in range(2):
                            ps = pspool.tile([P, GRP], f32, name=f"ps{dh}",
                                             tag=f"ps{dh}")
                            for s0 in range(0, Eg, SUB):
                                E = min(SUB, Eg - s0)
                                for ch in range(2):
                                    wc = ((t * 2 + ch) * 2 + dh) * P
                                    nc.tensor.matmul(
                                        ps[:, s0:s0 + E],
                                        wsb[:, wc:wc + P],
                                        xh[ch][:, g0 + s0:g0 + s0 + E],
                                        start=(ch == 0), stop=(ch == 1),
                                    )
                            bcol = t * 2 + dh
                            if dh == 0:
                                nc.vector.tensor_scalar(
                                    out=yh[dh][:, g0:g0 + Eg],
                                    in0=ps[:, :Eg],
                                    scalar1=bsb[:, bcol:bcol + 1],
                                    scalar2=None,
                                    op0=add,
                                )
                            else:
                                nc.scalar.activation(
                                    out=yh[dh][:, g0:g0 + Eg],
                                    in_=ps[:, :Eg],
                                    func=copy_fn,
                                    bias=bsb[:, bcol:bcol + 1],
                                )
                    for dh in range(2):
                        nc.sync.dma_start(
                            out=y[dh, :, off:off + Gt], in_=yh[dh][:, :Gt]
                        )

    nc.compile()
    return nc


def _plan(ids):
    """Shared sharding plan: returns (core_idx, offs, G, NP)."""
    ids = np.asarray(ids)
    order = np.argsort(ids, kind="stable")
    core_idx = [order[k::N_CORES] for k in range(N_CORES)]
    cnts = np.stack(
        [np.bincount(np.clip(ids[ci], 0, T - 1), minlength=T)[:T]
         for ci in core_idx]
    )
    # count only in-range types
    cnts = np.stack(
        [np.bincount(ids[ci][(ids[ci] >= 0) & (ids[ci] < T)].astype(np.int64),
                     minlength=T)[:T] for ci in core_idx]
    )
    gmax = cnts.max(axis=0)
    G = ((gmax + ALIGN - 1) // ALIGN) * ALIGN
    offs = np.concatenate([[0], np.cumsum(G)]).astype(np.int64)
    return core_idx, offs, tuple(int(g) for g in G), int(G.sum())


def _pack_inputs(x, w, b, ids, core_idx, offs, NP):
    """Build per-core device input maps (shared with the timing harness)."""
    import ml_dtypes

    bf16 = ml_dtypes.bfloat16
    # w_dev[c, t, ch, dh, dc] = W[t, dh*128+dc, ch*128+c]
    w_dev = np.ascontiguousarray(
        w.reshape(T, 2, P, 2, P).transpose(4, 0, 3, 1, 2).reshape(P, T * 4 * P)
    ).astype(bf16)
    # bias_dev[p, t*2+dh] = b[t, dh*128+p]
    b_dev = np.ascontiguousarray(
        b.reshape(T, 2, P).transpose(2, 0, 1).reshape(P, 2 * T)
    ).astype(np.float32)
    chain0 = np.zeros((1, 4), dtype=np.float32)

    in_maps = []
    seg_rows = []
    for k in range(N_CORES):
        ci = core_idx[k]
        ids_k = ids[ci]
        xr = np.zeros((NP, C), dtype=bf16)
        segs = []
        for t in range(T):
            idx_t = ci[ids_k == t]
            cnt = idx_t.shape[0]
            if cnt:
                xr[offs[t]:offs[t] + cnt] = x[idx_t].astype(bf16)
            segs.append((int(offs[t]), cnt, idx_t))
        seg_rows.append(segs)
        xt_k = np.ascontiguousarray(xr.T.reshape(2, P, NP))
        in_maps.append({
            "xt": xt_k,
            "w": w_dev,
            "bias": b_dev,
            "chain": chain0,
        })
    return in_maps, seg_rows


def kernel(edge_features, weights, biases, edge_type_ids):
    from concourse.bass_utils import run_bass_kernel_spmd

    x = np.ascontiguousarray(np.asarray(edge_features), dtype=np.float32)
    w = np.ascontiguousarray(np.asarray(weights), dtype=np.float32)
    b = np.ascontiguousarray(np.asarray(biases), dtype=np.float32)
    ids = np.asarray(edge_type_ids)
    n = x.shape[0]

    core_idx, offs, G, NP = _plan(ids)

    if G not in _cache:
        _cache[G] = _build_program(G)
    nc = _cache[G]

    in_maps, seg_rows = _pack_inputs(x, w, b, ids, core_idx, offs, NP)

    res = run_bass_kernel_spmd(nc, in_maps, list(range(N_CORES)))

    # zeros, not empty: rows whose type id falls outside [0, T) are never
    # written by any segment, and the reference leaves them at zero too
    out = np.zeros((n, D), dtype=np.float32)
    for k in range(N_CORES):
        yk = np.asarray(res.results[k]["y"]).reshape(D, NP)
        for off, cnt, idx_t in seg_rows[k]:
            if cnt:
                out[idx_t] = yk[:, off:off + cnt].T.astype(np.float32)
    return out
